# revision 13
# baseline (speedup 1.0000x reference)
"""ChildSum TreeLSTM (relational) — Trainium2 Bass kernel, 8 NeuronCores.

Strategy (data-parallel over batch, per sharding hint):
  - 16 trees are split over 8 cores, 2 whole trees per core.
  - Inside each core, nodes are relabeled level-by-level (sorted by tree
    height) so each bottom-up level occupies a contiguous row range of a
    padded node space.  All per-level gathers/scatters become small
    matmuls against host-built 0/1 incidence matrices (exact in fp).
  - Commits are UNMASKED full-ptile writes: the per-level gather uses a
    cumulative adjacency (children of all rows committed so far), and the
    fc term lives in a persistent PSUM accumulator per target ptile, so
    recomputing an already-committed row reproduces exactly its committed
    value (children h/c are stable once committed; incidence zeros kill
    contributions from rows that change).  Rows above the current level
    receive bounded garbage and are rewritten at their own level.
  - h state, gathered h^T, and the h-side weights (WiouH/Wfh) are bf16
    (halves LDWEIGHTS + DMA); c state, x projections, and everything the
    c accumulation touches stay fp32/f32r.
  - Gates are packed i|u|o so i and u (needed first, for c_new) come out
    of the first 512-wide PSUM chunk.
  - Embedding rows are gathered on-device with indirect DMA from the
    replicated emb table; rel rows via a one-hot matmul that lands
    directly transposed.  LSTM weights are replicated to every core.
  - Per-core output is the [12, trees_per_core] logits; the host
    assembles the [16, 12] result.

The SPMD program is identical on all cores; per-core behavior differs
only through input data (index vectors + incidence matrices).  Level
sizes are padded to the max across cores.
"""

import os
import numpy as np

P = 128
H = 256
HT = H // P          # h-state partition tiles
G3 = 3 * H           # packed i|u|o width (768)
N_CORES = 8


# ----------------------------------------------------------------------------
# Host-side plan builder
# ----------------------------------------------------------------------------

def _ceil_to(x, m):
    return (x + m - 1) // m * m


def _split_chunks(row0, cnt):
    """Split a row range into pieces that don't straddle 128-partition tiles."""
    out = []
    r, remaining = row0, cnt
    while remaining > 0:
        take = min(P - (r % P), remaining)
        out.append((r, take))
        r += take
        remaining -= take
    return out


def build_plan(xs, rels, child_idx, parent_idx, node_height, n_levels,
               n_cores=N_CORES):
    xs = np.asarray(xs)
    rels = np.asarray(rels)
    B, S = xs.shape
    tpc = B // n_cores
    heights = np.asarray(node_height).reshape(B, S)
    ci = np.asarray(child_idx)
    pi = np.asarray(parent_idx)
    NL = min(int(heights.max()) + 1, int(n_levels))

    edges_by_parent = {}
    for c, p in zip(ci.tolist(), pi.tolist()):
        edges_by_parent.setdefault(p, []).append(c)

    core_nodes, core_edges = [], []
    for core in range(n_cores):
        nl = [[] for _ in range(NL)]
        el = [[] for _ in range(NL)]
        for t in range(tpc):
            b = core * tpc + t
            for s in range(S):
                h = int(heights[b, s])
                if h < NL:
                    nl[h].append((t, s))
        for lv in range(1, NL):
            for (t, s) in nl[lv]:
                pg = (core * tpc + t) * S + s
                for cg in edges_by_parent.get(pg, []):
                    el[lv].append((cg, pg))
        core_nodes.append(nl)
        core_edges.append(el)

    n_hat = [max(len(core_nodes[c][lv]) for c in range(n_cores)) for lv in range(NL)]
    e_hat = [max(len(core_edges[c][lv]) for c in range(n_cores)) for lv in range(NL)]
    n_off = [0]
    for v in n_hat:
        n_off.append(n_off[-1] + v)
    e_off = [0]
    for v in e_hat:
        e_off.append(e_off[-1] + v)
    Npad = max(P, _ceil_to(n_off[-1], P))
    Epad = max(P, _ceil_to(e_off[-1], P))
    NKT, NET = Npad // P, Epad // P

    edge_chunks = [_split_chunks(e_off[lv], e_hat[lv]) for lv in range(NL)]
    # target node ptiles per level
    kts = [sorted({r // P for (r, c) in _split_chunks(n_off[lv], n_hat[lv])})
           for lv in range(NL)]

    # fc PSUM accumulator schedule: for each target ptile, the first/last
    # edge level that scatters into it
    fc_first, fc_last = {}, {}
    for lv in range(1, NL):
        for kN in kts[lv]:
            fc_first.setdefault(kN, lv)
            fc_last[kN] = lv
    fc_ptiles = sorted(fc_first)

    # packed per-(level, edge-chunk, target-ptile) scatter blocks:
    # rows = chunk-local edge, cols = full 128 node slots of the target ptile
    afc_col = {}
    ac = 0
    for lv in range(1, NL):
        for ec_i in range(len(edge_chunks[lv])):
            for kN in kts[lv]:
                afc_col[(lv, ec_i, kN)] = ac
                ac += P
    AC = max(ac, 1)

    per_core = []
    for core in range(n_cores):
        slot_of = {}
        xs_idx = np.zeros((Npad, 1), np.int32)
        rel_idx = np.zeros((Npad, 1), np.int32)
        for lv in range(NL):
            for j, (t, s) in enumerate(core_nodes[core][lv]):
                slot = n_off[lv] + j
                g = (core * tpc + t) * S + s
                slot_of[g] = slot
                b = core * tpc + t
                xs_idx[slot, 0] = xs[b, s]
                rel_idx[slot, 0] = rels[b, s]
        G = np.zeros((NKT, P, Epad), np.float32)
        Gp = np.zeros((NKT, P, Epad), np.float32)
        Adj = np.zeros((NKT, P, Npad), np.float32)
        AfcL = np.zeros((P, AC), np.float32)
        Pperm = np.zeros((NKT, P, tpc * S), np.float32)
        for lv in range(1, NL):
            for j, (cg, pg) in enumerate(core_edges[core][lv]):
                e = e_off[lv] + j
                cs, ps = slot_of[cg], slot_of[pg]
                G[cs // P, cs % P, e] = 1.0
                Gp[ps // P, ps % P, e] = 1.0
                Adj[cs // P, cs % P, ps] = 1.0
                for ci_, (erow, ecnt) in enumerate(edge_chunks[lv]):
                    if erow <= e < erow + ecnt:
                        a0 = afc_col[(lv, ci_, ps // P)]
                        AfcL[e - erow, a0 + ps % P] = 1.0
                        break
        for g, slot in slot_of.items():
            t = g // S - core * tpc
            s = g % S
            Pperm[slot // P, slot % P, t * S + s] = 1.0
        per_core.append(dict(xs_idx=xs_idx, rel_idx=rel_idx, G=G, Gp=Gp,
                             Adj=Adj, AfcL=AfcL, Pperm=Pperm))

    # SPMD-uniform nonzero-block flags (OR across cores), level-exact columns
    gnz = np.zeros((NL, NKT), bool)
    for lv in range(1, NL):
        esl = slice(e_off[lv], e_off[lv] + e_hat[lv])
        for k in range(NKT):
            gnz[lv, k] = any(per_core[c]["G"][k, :, esl].any()
                             for c in range(n_cores))
    gpnz = np.zeros((NET, NKT), bool)
    for ke in range(NET):
        esl = slice(ke * P, (ke + 1) * P)
        for k in range(NKT):
            gpnz[ke, k] = any(per_core[c]["Gp"][k, :, esl].any()
                              for c in range(n_cores))

    # combined gather blocks per level (>=1):
    # cols = [G-cols (even-padded) | delta-Adj cols of each target ptile].
    # The delta-Adj rows are restricted to the nodes committed at the
    # PREVIOUS level (or all earlier nodes at a ptile's first level): the
    # per-target h_sum @ WiouH lives in a persistent PSUM accumulator that
    # receives exactly one contribution per child, at the level right
    # after that child commits.
    ga_off, ga_w, ga_ec2 = {}, {}, {}
    ga_total = 0
    for lv in range(1, NL):
        ec2 = e_hat[lv] + (e_hat[lv] & 1)
        w = ec2 + P * len(kts[lv])
        assert w <= 512, f"level {lv} gather block too wide ({w})"
        ga_ec2[lv] = ec2
        ga_off[lv] = ga_total
        ga_w[lv] = w
        ga_total += w
    for cd in per_core:
        GA = np.zeros((NKT, P, max(ga_total, 2)), np.float32)
        for lv in range(1, NL):
            o0, ec2 = ga_off[lv], ga_ec2[lv]
            e0 = e_off[lv]
            for k in range(NKT):
                GA[k, :, o0:o0 + min(ec2, Epad - e0)] = \
                    cd["G"][k][:, e0:e0 + min(ec2, Epad - e0)]
                for i, kN in enumerate(kts[lv]):
                    blk = cd["Adj"][k][:, kN * P:(kN + 1) * P].copy()
                    r_lo = 0 if fc_first[kN] == lv else n_off[lv - 1]
                    r_hi = n_off[lv]
                    lo_l = max(r_lo - k * P, 0)
                    hi_l = max(min(r_hi - k * P, P), 0)
                    blk[:lo_l, :] = 0.0
                    blk[hi_l:, :] = 0.0
                    GA[k, :, o0 + ec2 + i * P:o0 + ec2 + (i + 1) * P] = blk
        cd["GA"] = GA
    GAtot = max(ga_total, 2)
    kgb = np.zeros((NL, NKT), bool)
    for lv in range(1, NL):
        for k in range(NKT):
            kgb[lv, k] = any(
                per_core[c]["GA"][k][:, ga_off[lv]:ga_off[lv] + ga_w[lv]].any()
                for c in range(n_cores))

    sizes = dict(NL=NL, Npad=Npad, Epad=Epad, NKT=NKT, NET=NET, tpc=tpc, S=S,
                 AC=AC, n_hat=n_hat, e_hat=e_hat, n_off=n_off,
                 e_off=e_off, edge_chunks=edge_chunks, kts=kts,
                 afc_col=afc_col, gnz=gnz, gpnz=gpnz, kgb=kgb,
                 ga_off=ga_off, ga_w=ga_w, ga_ec2=ga_ec2, GAtot=GAtot,
                 fc_first=fc_first, fc_last=fc_last, fc_ptiles=fc_ptiles)

    # ---- packed constant column layouts ----
    TS = tpc * S
    cols = {}          # f32 block
    cptr = 0
    def _alloc(name, w):
        nonlocal cptr
        cols[name] = (cptr, w)
        cptr += w
    _alloc("bias", G3 + H + 16)       # row0: [bi512 | bf256 | bout]
    _alloc("ones", P)
    _alloc("relw64", P)
    _alloc("relh", Npad)
    _alloc("ident", P)
    for d in range(2):
        _alloc(f"wioux{d}", G3)
        _alloc(f"wfx{d}", H)
    for k2 in range(2):
        _alloc(f"wout{k2}", 16)
    for k in range(NKT):
        _alloc(f"G{k}", Epad)
    for k in range(NKT):
        _alloc(f"Gp{k}", Epad)
    _alloc("Afc", AC + (AC & 1))
    sizes["cols"] = cols
    sizes["C"] = cptr

    bcols = {}         # bf16 block
    bptr = 0
    def _balloc(name, w):
        nonlocal bptr
        bcols[name] = (bptr, w)
        bptr += w
    for k2 in range(2):
        _balloc(f"wiouh{k2}", G3)
        _balloc(f"wfh{k2}", H)
    for k in range(NKT):
        _balloc(f"GA{k}", GAtot)
    for k in range(NKT):
        _balloc(f"Pp{k}", TS)
    sizes["bcols"] = bcols
    sizes["CB"] = bptr

    icols = {}
    iptr = 0
    def _ialloc(name, w):
        nonlocal iptr
        icols[name] = (iptr, w)
        iptr += w
    _ialloc("xsidx", NKT)
    _ialloc("relidx", NKT)
    sizes["icols"] = icols
    sizes["CI"] = iptr
    return sizes, per_core


def pack_weights(inp):
    f32 = np.float32
    a = lambda k: np.asarray(inp[k], f32)
    # gate order i | u | o
    WiouX = np.ascontiguousarray(
        np.concatenate([a("W_ix"), a("W_ux"), a("W_ox")], axis=1))   # [DIN,768]
    WiouH = np.ascontiguousarray(
        np.concatenate([a("W_ih"), a("W_uh"), a("W_oh")], axis=1))   # [H,768]
    bi512 = np.zeros((1, 512), f32)
    bi512[0, :H] = a("b_ix") + a("b_ih")
    bf = np.ascontiguousarray((a("b_fx") + a("b_fh")).reshape(1, H))
    return WiouX, WiouH, bi512, bf


# ----------------------------------------------------------------------------
# Numpy emulation of the device program (validation / fallback)
# ----------------------------------------------------------------------------

def _bf16(x):
    import ml_dtypes
    return np.asarray(x, np.float32).astype(ml_dtypes.bfloat16).astype(np.float32)


def emulate_core(sizes, cd, emb_W, rel_W, WiouX, WiouH, Wfx, Wfh,
                 bi512, bf, Wout, bout, quant=True):
    """Mirror of the device program, incl. unmasked commits, persistent fc
    accumulation, and (optionally) bf16 quantization of the h path."""
    f32 = np.float32
    q = _bf16 if quant else (lambda x: x)
    NL, NKT = sizes["NL"], sizes["NKT"]
    Npad = sizes["Npad"]
    x = np.concatenate([emb_W[cd["xs_idx"][:, 0]], rel_W[cd["rel_idx"][:, 0]]],
                       axis=1).astype(f32)
    iou_x = (x @ WiouX).astype(f32)
    iou_x[:, :512] += bi512[0]
    fx = (x @ Wfx).astype(f32) + bf[0]
    GpF = np.concatenate([cd["Gp"][k] for k in range(NKT)], axis=0)
    fxe = (GpF.T @ fx).astype(f32)
    GF = np.concatenate([cd["G"][k] for k in range(NKT)], axis=0)
    GAF = np.concatenate([cd["GA"][k] for k in range(NKT)], axis=0)
    WiouHq = q(WiouH)
    Wfhq = q(Wfh)

    h = np.zeros((Npad, H), f32)     # bf16-committed values
    c = np.zeros((Npad, H), f32)
    fc_acc = {kN: np.zeros((P, H), f32) for kN in sizes["fc_ptiles"]}
    iou_acc = {}

    def sigmoid(v):
        return (1.0 / (1.0 + np.exp(-v.astype(f32)))).astype(f32)

    for lv in range(NL):
        kts = sizes["kts"][lv]
        if lv > 0:
            o0, ec2, gaw = sizes["ga_off"][lv], sizes["ga_ec2"][lv], sizes["ga_w"][lv]
            hgst = q((q(h).T @ GAF[:, o0:o0 + gaw]).astype(f32))   # [H, gaw]
            for ec_i, (erow, ecnt) in enumerate(sizes["edge_chunks"][lv]):
                eloc = erow - sizes["e_off"][lv]
                cg = (GF[:, erow:erow + ecnt].T @ c).astype(f32)
                fpre = (hgst[:, eloc:eloc + ecnt].T @ Wfhq).astype(f32) \
                    + fxe[erow:erow + ecnt]
                fce = np.zeros((P, H), f32)
                fce[:ecnt] = (sigmoid(fpre) * cg).astype(f32)
                for kN in kts:
                    a0 = sizes["afc_col"][(lv, ec_i, kN)]
                    Af = cd["AfcL"][:, a0:a0 + P]
                    fc_acc[kN] += (Af.T @ fce).astype(f32)
        for kti, kN in enumerate(kts):
            if lv > 0:
                hoff = sizes["ga_ec2"][lv] + kti * P
                if sizes["fc_first"][kN] == lv:
                    iou_acc[kN] = iou_x[kN * P:(kN + 1) * P].copy()
                iou_acc[kN] = (iou_acc[kN]
                               + (hgst[:, hoff:hoff + P].T @ WiouHq)).astype(f32)
                iou = iou_acc[kN]
            else:
                iou = iou_x[kN * P:(kN + 1) * P].copy()
            i = sigmoid(iou[:, 0:H])
            u = np.tanh(iou[:, H:2 * H]).astype(f32)
            og = sigmoid(iou[:, 2 * H:])
            cn = (i * u).astype(f32)
            if lv >= sizes["fc_first"].get(kN, 99):
                cn = (cn + fc_acc[kN]).astype(f32)
            hn = q((og * np.tanh(cn)).astype(f32))
            c[kN * P:(kN + 1) * P] = cn
            h[kN * P:(kN + 1) * P] = hn

    PF = np.concatenate([cd["Pperm"][k] for k in range(NKT)], axis=0)
    hT_ord = (q(h).T @ PF).astype(f32)
    S = sizes["S"]
    pooled = np.stack([hT_ord[:, t * S:(t + 1) * S].max(axis=1)
                       for t in range(sizes["tpc"])], axis=1)
    return (Wout.T @ pooled).astype(f32) + bout[:, None]      # [12, tpc]


def kernel_numpy(**inputs):
    sizes, per_core = build_plan(inputs["xs"], inputs["rels"],
                                 inputs["child_idx"], inputs["parent_idx"],
                                 inputs["node_height"], int(inputs["n_levels"]))
    WiouX, WiouH, bi512, bf = pack_weights(inputs)
    emb_W = np.asarray(inputs["emb_W"], np.float32)
    rel_W = np.asarray(inputs["rel_W"], np.float32)
    outs = []
    for cd in per_core:
        lT = emulate_core(sizes, cd, emb_W, rel_W, WiouX, WiouH,
                          np.asarray(inputs["W_fx"], np.float32),
                          np.asarray(inputs["W_fh"], np.float32),
                          bi512, bf,
                          np.asarray(inputs["W_out"], np.float32),
                          np.asarray(inputs["b_out"], np.float32))
        outs.append(lT.T)
    return np.concatenate(outs, axis=0).astype(np.float32)


# ----------------------------------------------------------------------------
# Device program
# ----------------------------------------------------------------------------

def build_bass(sizes, V, DE, RV, DR, L):
    from concourse import bacc, bass, mybir, tile

    f32 = mybir.dt.float32
    f32r = mybir.dt.float32r
    bf16 = mybir.dt.bfloat16
    i32 = mybir.dt.int32
    SIG = mybir.ActivationFunctionType.Sigmoid
    TANH = mybir.ActivationFunctionType.Tanh
    AXX = mybir.AxisListType.X

    NL, Npad, Epad = sizes["NL"], sizes["Npad"], sizes["Epad"]
    NKT, NET, tpc, S = sizes["NKT"], sizes["NET"], sizes["tpc"], sizes["S"]
    C, CB, CI = sizes["C"], sizes["CB"], sizes["CI"]
    cols, bcols, icols = sizes["cols"], sizes["bcols"], sizes["icols"]
    DIN = DE + DR
    DT = DIN // P
    TS = tpc * S
    mgw = max(sizes["ga_w"].values()) if sizes["ga_w"] else 2
    assert mgw <= 256, f"gather block too wide for single-bank psum ({mgw})"

    nc = bacc.Bacc("TRN2", target_bir_lowering=False, debug=False)

    d_emb = nc.dram_tensor("emb", [V, DE], f32, kind="ExternalInput")
    d_rel = nc.dram_tensor("relw", [RV, DR], f32, kind="ExternalInput")
    d_bigc = nc.dram_tensor("bigc", [P, C], f32, kind="ExternalInput")
    d_bigb = nc.dram_tensor("bigb", [P, CB], bf16, kind="ExternalInput")
    d_idf = nc.dram_tensor("identf", [P, P], f32, kind="ExternalInput")
    d_bigi = nc.dram_tensor("bigi", [P, max(CI, 1)], i32, kind="ExternalInput")
    d_out = nc.dram_tensor("out", [L, tpc], f32, kind="ExternalOutput")

    with tile.TileContext(nc) as tc:
        with (
            tc.tile_pool(name="const", bufs=1) as cp,
            tc.tile_pool(name="psg", bufs=2, space="PSUM") as ps_g,
            tc.tile_pool(name="pscg", bufs=1, space="PSUM") as ps_cg,
            tc.tile_pool(name="psfp", bufs=2, space="PSUM") as ps_fp,
            tc.tile_pool(name="psiou", bufs=1, space="PSUM") as ps_iou,
            tc.tile_pool(name="psfc", bufs=1, space="PSUM") as ps_fc,
        ):
            t = lambda shape, dt_, tag: cp.tile(shape, dt_, tag=tag, name=tag)
            bigc = t([P, C], f32r, "bigc")
            bigb = t([P, CB], bf16, "bigb")
            bigi = t([P, max(CI, 1)], i32, "bigi")

            def cc(name):
                off, w = cols[name]
                return bigc[:, off:off + w]

            def cb(name):
                off, w = bcols[name]
                return bigb[:, off:off + w]

            def ci(name, j):
                off, _ = icols[name]
                return bigi[:, off + j:off + j + 1]

            wioux = [cc(f"wioux{d}") for d in range(DT)]
            wfx = [cc(f"wfx{d}") for d in range(DT)]
            wiouh = [cb(f"wiouh{k}") for k in range(HT)]
            wfh = [cb(f"wfh{k}") for k in range(HT)]
            wout = [cc(f"wout{k}")[:, :L] for k in range(HT)]
            boff = cols["bias"][0]
            bi_row = bigc[0:1, boff:boff + 512]
            bf_row = bigc[0:1, boff + G3:boff + G3 + H]
            bout_row = bigc[0:1, boff + G3 + H:boff + G3 + H + L]
            ones_row = bigc[0:1, cols["ones"][0]:cols["ones"][0] + P]
            identr = cc("ident")
            identf = t([P, P], f32, "identf")
            Gsb = [cc(f"G{k}") for k in range(NKT)]
            Gpsb = [cc(f"Gp{k}") for k in range(NKT)]
            Afcsb = cc("Afc")
            Ppsb = [cb(f"Pp{k}") for k in range(NKT)]

            xsall = t([P, NKT, DE], f32, "xsall")
            xT = [t([P, Npad], f32r, f"xT{d}") for d in range(DT)]
            ioux = [t([P, G3], f32r, f"ioux{k}") for k in range(NKT)]
            fxsb = [t([P, H], f32r, f"fx{k}") for k in range(NKT)]
            fxesb = [t([P, H], f32r, f"fxe{e}") for e in range(NET)]
            hrb = [t([P, H], bf16, f"h{k}") for k in range(NKT)]      # h state
            crb = [t([P, H], f32r, f"c{k}") for k in range(NKT)]      # c state
            hgst = t([P, HT, mgw], bf16, "hgst")
            fgate = t([P, H], f32, "fgate")
            fce = t([P, H], f32r, "fce")
            isb = t([P, H], f32, "isb")
            usb = t([P, H], f32, "usb")
            osb = t([P, H], bf16, "osb")
            cnew = t([P, H], f32, "cnew")
            thsb = t([P, H], bf16, "thsb")
            pooled = [t([P, tpc], f32r, f"pool{k}") for k in range(HT)]
            outsb = t([L, tpc], f32, "outsb")

            # ---- preamble loads, in need order; each dma lands on its own
            # queue slot (descriptors spread across the 16 HW queues)
            nc.sync.dma_start(bigi[:], d_bigi[:])
            rel_end = cols["ident"][0] + cols["ident"][1]
            nc.sync.dma_start(bigc[:, 0:rel_end],
                              d_bigc[:, 0:rel_end].bitcast(f32r))
            misc_end = cols["wfx1"][0] + cols["wfx1"][1]
            nc.sync.dma_start(bigc[:, rel_end:misc_end],
                              d_bigc[:, rel_end:misc_end].bitcast(f32r))
            nc.sync.dma_start(identf[:], d_idf[:])
            bw_end = bcols["GA0"][0]
            nc.sync.dma_start(bigb[:, 0:bw_end], d_bigb[:, 0:bw_end])
            ga_end = bcols[f"GA{NKT-1}"][0] + bcols[f"GA{NKT-1}"][1]
            nc.sync.dma_start(bigb[:, bw_end:ga_end], d_bigb[:, bw_end:ga_end])
            nc.sync.dma_start(bigc[:, misc_end:C],
                              d_bigc[:, misc_end:C].bitcast(f32r))
            nc.sync.dma_start(bigb[:, ga_end:CB], d_bigb[:, ga_end:CB])

            # ---- embedding gathers (per-ptile 2D indirect DMAs on gpsimd,
            # which has nothing else queued before these)
            for k in range(NKT):
                nc.gpsimd.indirect_dma_start(
                    out=xsall[:, k, :], out_offset=None, in_=d_emb[:],
                    in_offset=bass.IndirectOffsetOnAxis(ap=ci("xsidx", k), axis=0))

            # ---- state zero-init on DVE (off the gpsimd queue)
            for k in range(NKT):
                nc.vector.memzero(hrb[k][:])
                nc.vector.memzero(crb[k][:])
            nc.vector.memzero(fce[:])

            # ---- rel rows via one-hot matmul (lands transposed into xT[1])
            relw_off = cols["relw64"][0]
            relh_off = cols["relh"][0]
            prl = ps_cg.tile([P, Npad], f32, tag="cg", name="cg")
            nc.tensor.matmul(prl[:, :],
                             lhsT=bigc[0:RV, relw_off:relw_off + P],
                             rhs=bigc[0:RV, relh_off:relh_off + Npad],
                             start=True, stop=True)
            nc.vector.tensor_copy(out=xT[1][P - DR:P, :],
                                  in_=prl[P - DR:P, :])

            # ---- x transpose (emb part only: DE = 192 = 128 + 64 cols)
            for k in range(NKT):
                pt = ps_g.tile([P, 256], f32, tag="gst", name="gst")
                nc.tensor.transpose(pt[:, 0:P], xsall[:, k, 0:P], identf[:])
                nc.vector.tensor_copy(out=xT[0][:, k * P:(k + 1) * P],
                                      in_=pt[:, 0:P])
                pt2 = ps_g.tile([P, 256], f32, tag="gst", name="gst")
                nc.tensor.transpose(pt2[:DE - P, 0:P], xsall[:, k, P:DE],
                                    identf[:])
                nc.vector.tensor_copy(out=xT[1][0:DE - P, k * P:(k + 1) * P],
                                      in_=pt2[:DE - P, 0:P])

            # ---- input projections (biases folded in via ones-row matmul)
            fx_used = [any(sizes["gpnz"][ke, k] for ke in range(NET))
                       for k in range(NKT)]
            for k in range(NKT):
                pi = ps_iou.tile([P, G3], f32, tag="iou", name="iou")
                for c0, cn_ in ((0, 512), (512, G3 - 512)):
                    has_bias = (c0 == 0)
                    for d in range(DT):
                        nc.tensor.matmul(
                            pi[:, c0:c0 + cn_],
                            lhsT=xT[d][:, k * P:(k + 1) * P],
                            rhs=wioux[d][:, c0:c0 + cn_],
                            start=(d == 0),
                            stop=(not has_bias and d == DT - 1))
                    if has_bias:
                        nc.tensor.matmul(pi[:, 0:512], lhsT=ones_row,
                                         rhs=bi_row, start=False, stop=True)
                nc.vector.tensor_copy(out=ioux[k][:, 0:512], in_=pi[:, 0:512])
                nc.scalar.copy(out=ioux[k][:, 512:G3], in_=pi[:, 512:G3])
                if not fx_used[k]:
                    continue
                pf = ps_fp.tile([P, H], f32, tag="fp", name="fp")
                for d in range(DT):
                    nc.tensor.matmul(pf[:], lhsT=xT[d][:, k * P:(k + 1) * P],
                                     rhs=wfx[d][:], start=(d == 0), stop=False)
                nc.tensor.matmul(pf[:], lhsT=ones_row, rhs=bf_row,
                                 start=False, stop=True)
                nc.vector.tensor_copy(out=fxsb[k][:], in_=pf[:])

            # ---- fxe: fx gathered per edge slot
            for ke in range(NET):
                ks = [k for k in range(NKT) if sizes["gpnz"][ke, k]]
                if not ks:
                    nc.vector.memzero(fxesb[ke][:])
                    continue
                pf = ps_fp.tile([P, H], f32, tag="fp", name="fp")
                for i, k in enumerate(ks):
                    nc.tensor.matmul(pf[:],
                                     lhsT=Gpsb[k][:, ke * P:(ke + 1) * P],
                                     rhs=fxsb[k][:],
                                     start=(i == 0), stop=(i == len(ks) - 1))
                nc.vector.tensor_copy(out=fxesb[ke][:], in_=pf[:])

            # ---- persistent accumulators (allocated lazily at each target
            # ptile's first level; live ranges don't overlap, so both fc
            # accumulators share one PSUM bank)
            fc_acc = {}
            iou_acc = {}

            # ---- levels
            for lv in range(NL):
                kts = sizes["kts"][lv]
                if lv > 0:
                    prev = set(sizes["kts"][lv - 1])
                    okey = lambda k: (k in prev, k)
                    kg = sorted((k for k in range(NKT) if sizes["gnz"][lv, k]),
                                key=okey)
                    kgbl = sorted((k for k in range(NKT) if sizes["kgb"][lv, k]),
                                  key=okey)
                    echunks = sizes["edge_chunks"][lv]
                    ga0l = sizes["ga_off"][lv]
                    gawl = sizes["ga_w"][lv]
                    ec2 = sizes["ga_ec2"][lv]

                    # PE: c-gather first (depends only on prev level's crb,
                    # which commits ~1us before hrb)
                    pcs = []
                    for ec_i, (erow, ecnt) in enumerate(echunks):
                        pc = ps_cg.tile([P, H], f32, tag="cg", name="cg")
                        for i, k in enumerate(kg):
                            nc.tensor.matmul(
                                pc[:ecnt, :],
                                lhsT=Gsb[k][:, erow:erow + ecnt],
                                rhs=crb[k][:],
                                start=(i == 0), stop=(i == len(kg) - 1))
                        pcs.append(pc)

                    # PE: combined gather h_children^T | delta h_sum^T.
                    # One psum tile per H-half (a PSUM bank can host only
                    # one accumulation group at a time on HW), casts split
                    # so the iou/fpre consumers start as soon as possible
                    for kh in range(HT):
                        pg = ps_g.tile([P, 256], f32, tag="gst", name="gst")
                        for i, k in enumerate(kgbl):
                            nc.tensor.matmul(
                                pg[:, :gawl],
                                lhsT=hrb[k][:, kh * P:(kh + 1) * P],
                                rhs=bigb[:, bcols[f"GA{k}"][0] + ga0l:
                                         bcols[f"GA{k}"][0] + ga0l + gawl],
                                start=(i == 0), stop=(i == len(kgbl) - 1))
                        nc.vector.tensor_copy(out=hgst[:, kh, :gawl],
                                              in_=pg[:, :gawl])

                    # PE: f preactivation = h_ch @ Wfh + fxe (bias in fxe)
                    pfps = []
                    for ec_i, (erow, ecnt) in enumerate(echunks):
                        ke, r0e = erow // P, erow % P
                        eloc = erow - sizes["e_off"][lv]
                        pfp = ps_fp.tile([P, H], f32, tag="fp", name="fp")
                        for kh in range(HT):
                            nc.tensor.matmul(pfp[:ecnt, :],
                                             lhsT=hgst[:, kh, eloc:eloc + ecnt],
                                             rhs=wfh[kh][:],
                                             start=(kh == 0), stop=False)
                        nc.tensor.matmul(pfp[:ecnt, :],
                                         lhsT=identr[:, r0e:r0e + ecnt],
                                         rhs=fxesb[ke][:],
                                         start=False, stop=True)
                        pfps.append(pfp)

                # i/u chunk of iou per target ptile (i and u feed c_new);
                # the iou preactivation lives in a persistent PSUM
                # accumulator: ioux is added once (ptile-first level), then
                # each level adds only the delta h_sum @ WiouH of the
                # children committed at the previous level
                for kti, kN in enumerate(kts):
                    if lv > 0:
                        first = (sizes["fc_first"][kN] == lv)
                        last = (sizes["fc_last"][kN] == lv)
                        hoff = sizes["ga_ec2"][lv] + kti * P
                        if first:
                            pi = ps_iou.tile([P, G3], f32, tag="iou",
                                             name=f"iou{kN}")
                            iou_acc[kN] = pi
                        pi = iou_acc[kN]
                        if first:
                            nc.tensor.matmul(
                                pi[:, 0:512], lhsT=identr[:, :P],
                                rhs=ioux[kN][:, 0:512],
                                start=True, stop=False,
                                skip_group_check=True)
                        for kh in range(HT):
                            nc.tensor.matmul(
                                pi[:, 0:512],
                                lhsT=hgst[:, kh, hoff:hoff + P],
                                rhs=wiouh[kh][:, 0:512],
                                start=False, stop=(last and kh == HT - 1),
                                skip_group_check=True)

                if lv > 0:
                    # scalar: f gate; DVE: fce; PE: scatter into fc_acc
                    for ec_i, (erow, ecnt) in enumerate(echunks):
                        nc.scalar.activation(fgate[:ecnt, :],
                                             pfps[ec_i][:ecnt, :], SIG)
                        nc.vector.tensor_mul(fce[:ecnt, :],
                                             fgate[:ecnt, :], pcs[ec_i][:ecnt, :])
                        for kN in kts:
                            if kN not in fc_acc:
                                fc_acc[kN] = ps_fc.tile([P, H], f32, tag="fc",
                                                        name=f"fc{kN}")
                            a0 = sizes["afc_col"][(lv, ec_i, kN)]
                            nc.tensor.matmul(
                                fc_acc[kN][:],
                                lhsT=Afcsb[:, a0:a0 + P],
                                rhs=fce[:],
                                start=(lv == sizes["fc_first"][kN] and ec_i == 0),
                                stop=(lv == sizes["fc_last"][kN]
                                      and ec_i == len(echunks) - 1),
                                skip_group_check=True)

                    # PE: o chunk of iou (same persistent accumulation)
                    for kti, kN in enumerate(kts):
                        first = (sizes["fc_first"][kN] == lv)
                        last = (sizes["fc_last"][kN] == lv)
                        hoff = sizes["ga_ec2"][lv] + kti * P
                        pi = iou_acc[kN]
                        if first:
                            nc.tensor.matmul(
                                pi[:, 512:G3], lhsT=identr[:, :P],
                                rhs=ioux[kN][:, 512:G3],
                                start=True, stop=False,
                                skip_group_check=True)
                        for kh in range(HT):
                            nc.tensor.matmul(
                                pi[:, 512:G3],
                                lhsT=hgst[:, kh, hoff:hoff + P],
                                rhs=wiouh[kh][:, 512:G3],
                                start=False, stop=(last and kh == HT - 1),
                                skip_group_check=True)

                # activations + unmasked commits
                for kti, kN in enumerate(kts):
                    if lv > 0:
                        pi = iou_acc[kN]
                        nc.scalar.activation(isb[:], pi[:, 0:H], SIG)
                        nc.scalar.activation(usb[:], pi[:, H:512], TANH)
                        nc.scalar.activation(osb[:], pi[:, 512:G3], SIG)
                    else:
                        iax = ioux[kN][:].bitcast(f32)
                        nc.scalar.activation(isb[:], iax[:, 0:H], SIG)
                        nc.scalar.activation(usb[:], iax[:, H:512], TANH)
                        nc.scalar.activation(osb[:], iax[:, 512:G3], SIG)
                    # c/h commits split into 128-col halves: the next
                    # level's gather for kh0 starts while kh1 still commits
                    has_fc = lv >= sizes["fc_first"].get(kN, NL + 1)
                    for hh in range(HT):
                        hs = slice(hh * P, (hh + 1) * P)
                        if has_fc:
                            nc.vector.tensor_mul(cnew[:, hs],
                                                 isb[:, hs], usb[:, hs])
                            nc.vector.tensor_add(crb[kN][:, hs], cnew[:, hs],
                                                 fc_acc[kN][:, hs])
                        else:
                            nc.vector.tensor_mul(crb[kN][:, hs],
                                                 isb[:, hs], usb[:, hs])
                        nc.scalar.activation(thsb[:, hs],
                                             crb[kN][:, hs].bitcast(f32), TANH)
                        nc.vector.tensor_mul(hrb[kN][:, hs],
                                             osb[:, hs], thsb[:, hs])

            # ---- readout
            plg = ps_fp.tile([P, tpc], f32, tag="fp", name="fp")
            last_kts = set(sizes["kts"][NL - 1])
            ro_order = sorted(range(NKT), key=lambda k: (k in last_kts, k))
            for kh in range(HT):
                pr = ps_cg.tile([P, TS], f32, tag="cg", name="cg")
                for i, k in enumerate(ro_order):
                    nc.tensor.matmul(pr[:],
                                     lhsT=hrb[k][:, kh * P:(kh + 1) * P],
                                     rhs=Ppsb[k][:],
                                     start=(i == 0), stop=(i == NKT - 1))
                for t_ in range(tpc):
                    nc.vector.reduce_max(pooled[kh][:, t_:t_ + 1],
                                         pr[:, t_ * S:(t_ + 1) * S], axis=AXX)
            for kh in range(HT):
                nc.tensor.matmul(plg[:L, :], lhsT=wout[kh],
                                 rhs=pooled[kh][:],
                                 start=(kh == 0), stop=False)
            nc.tensor.matmul(plg[:L, :], lhsT=bout_row,
                             rhs=ones_row[:, :tpc], start=False, stop=True)
            nc.vector.tensor_copy(out=outsb[:], in_=plg[:L, :])
            nc.sync.dma_start(d_out[:, :], outsb[:])

    nc.compile()
    return nc


def _make_in_maps(sizes, per_core, inputs):
    import ml_dtypes
    f32 = np.float32
    WiouX, WiouH, bi512, bf = pack_weights(inputs)
    cols, C = sizes["cols"], sizes["C"]
    bcols, CB = sizes["bcols"], sizes["CB"]
    icols, CI = sizes["icols"], sizes["CI"]
    NKT = sizes["NKT"]
    L = np.asarray(inputs["W_out"]).shape[1]

    base = np.zeros((P, C), f32)

    def put(name, arr, row0=0):
        off, w = cols[name]
        arr = np.asarray(arr, f32)
        base[row0:row0 + arr.shape[0], off:off + arr.shape[1]] = arr

    for d in range(2):
        put(f"wioux{d}", WiouX[d * P:(d + 1) * P])
        put(f"wfx{d}", np.asarray(inputs["W_fx"], f32)[d * P:(d + 1) * P])
    for k2 in range(2):
        put(f"wout{k2}", np.asarray(inputs["W_out"], f32)[k2 * P:(k2 + 1) * P])
    brow = np.zeros((1, cols["bias"][1]), f32)
    brow[0, :512] = bi512[0]
    brow[0, G3:G3 + H] = bf[0]
    brow[0, G3 + H:G3 + H + L] = np.asarray(inputs["b_out"], f32)
    put("bias", brow)
    put("ones", np.ones((1, P), f32))
    put("ident", np.eye(P, dtype=f32))
    relW = np.asarray(inputs["rel_W"], f32)          # [R, DR]
    rw = np.zeros((relW.shape[0], P), f32)
    rw[:, P - relW.shape[1]:] = relW                 # rel dims land at rows 64:128
    put("relw64", rw)

    bbase = np.zeros((P, CB), f32)

    def bput(name, arr, row0=0):
        off, w = bcols[name]
        arr = np.asarray(arr, f32)
        bbase[row0:row0 + arr.shape[0], off:off + arr.shape[1]] = arr

    for k2 in range(2):
        bput(f"wiouh{k2}", WiouH[k2 * P:(k2 + 1) * P])
        bput(f"wfh{k2}", np.asarray(inputs["W_fh"], f32)[k2 * P:(k2 + 1) * P])

    ibase = np.zeros((P, max(CI, 1)), np.int32)

    in_maps = []
    for cd in per_core:
        bc = base.copy()
        for k in range(NKT):
            off, w = cols[f"G{k}"]
            bc[:, off:off + w] = cd["G"][k]
            off, w = cols[f"Gp{k}"]
            bc[:, off:off + w] = cd["Gp"][k]
        off, w = cols["Afc"]
        bc[:, off:off + cd["AfcL"].shape[1]] = cd["AfcL"]
        off, w = cols["relh"]
        rh = np.zeros((P, w), f32)
        rh[cd["rel_idx"][:, 0], np.arange(w)] = 1.0
        bc[:, off:off + w] = rh
        bb = bbase.copy()
        for k in range(NKT):
            off, w = bcols[f"GA{k}"]
            bb[:, off:off + cd["GA"].shape[2]] = cd["GA"][k]
            off, w = bcols[f"Pp{k}"]
            bb[:, off:off + w] = cd["Pperm"][k]
        bi_ = ibase.copy()
        xo = icols["xsidx"][0]
        ro = icols["relidx"][0]
        for k in range(NKT):
            bi_[:, xo + k] = cd["xs_idx"][k * P:(k + 1) * P, 0]
            bi_[:, ro + k] = cd["rel_idx"][k * P:(k + 1) * P, 0]
        in_maps.append(dict(
            emb=np.ascontiguousarray(np.asarray(inputs["emb_W"], f32)),
            relw=np.ascontiguousarray(np.asarray(inputs["rel_W"], f32)),
            bigc=np.ascontiguousarray(bc),
            bigb=np.ascontiguousarray(bb.astype(ml_dtypes.bfloat16)),
            bigi=np.ascontiguousarray(bi_),
            identf=np.eye(P, dtype=f32),
        ))
    return in_maps


def kernel(**inputs):
    sizes, per_core = build_plan(inputs["xs"], inputs["rels"],
                                 inputs["child_idx"], inputs["parent_idx"],
                                 inputs["node_height"], int(inputs["n_levels"]))
    V, DE = np.asarray(inputs["emb_W"]).shape
    RV, DR = np.asarray(inputs["rel_W"]).shape
    L = np.asarray(inputs["W_out"]).shape[1]
    nc = build_bass(sizes, V, DE, RV, DR, L)
    in_maps = _make_in_maps(sizes, per_core, inputs)

    if os.environ.get("TREELSTM_SIM") == "1":
        from concourse.bass_interp import CoreSim
        outs = []
        for cid in range(N_CORES):
            sim = CoreSim(nc)
            for name, val in in_maps[cid].items():
                sim.tensor(name)[:] = val
            sim.simulate()
            outs.append(np.array(sim.tensor("out")).T)
        return np.concatenate(outs, axis=0).astype(np.float32)

    from concourse.bass_utils import run_bass_kernel_spmd
    res = run_bass_kernel_spmd(nc, in_maps, core_ids=list(range(N_CORES)),
                               trace=bool(int(os.environ.get("TREELSTM_TRACE", "0"))))
    if getattr(kernel, "_keep_results", False):
        kernel.last_results = res
    out = np.concatenate([r["out"].T for r in res.results], axis=0)
    return out.astype(np.float32)
